# revision 3
# baseline (speedup 1.0000x reference)
"""BiLSTM (T=256, B=64, NIN=H=NOUT=512) Trainium2 kernel over 8 NeuronCores.

Sharding: direction (2) x batch-quarter (4) = 8 cores, SPMD (one program).
Each core runs one direction's LSTM for 16 batch rows (backward cores get
time-reversed x), then computes its half of the final FC:
    out = h_f @ fc_w[:, :H].T + h_b @ fc_w[:, H:].T + fc_b
The host sums the two partial FC outputs per batch quarter. No collectives.

V2 changes vs baseline:
  - Recurrence matmuls in fp8(e4m3) with DoubleRow perf mode: contraction
    256 per instruction -> 32 gate MMs/step instead of 64. Weights scaled
    x32 (avoids e4m3 subnormals); the sigmoid ACT op applies scale=1/32.
  - Sigmoid-only cell math (no tanh table):
      * g-gate rows pre-scaled x2 so sigma(2 zg) = (tanh(zg)+1)/2
      * stored state h' = h/2 with W_hh / fc_w pre-scaled x2
      * t1h = (Gg-0.5)*Gi, c = 2*t1h + Gf*c_prev, u = sigma(2c),
        h' = (u-0.5)*Go  -- each one fused scalar_tensor_tensor DVE op.
  - PSUM layout [f][i|g][o]: i,g share one bank so sigma(i,g) is a single
    merged ACT op (the spine's first activation).
  - h' written twice: fp8 copy for the recurrence rhs, bf16 copy for FC
    (fp8 h into FC fails the accuracy budget; bf16 copy passes at 7e-3).
  - xg/FC stuffer matmuls emitted AFTER each step's gate MMs so they fill
    the ACT/DVE spine window instead of delaying the gate matmuls.
"""

import numpy as np

T, B, NIN, H, NOUT = 256, 64, 512, 512, 512
BL = B // 4          # local batch per core (batch quarter)
KT = H // 128        # 4 k-tiles over the hidden/contraction dim
MT = (4 * H) // 128  # 16 m-tiles over the gate dim
# PyTorch gate blocks [i,f,g,o] -> our order [f,i,g,o]
GATE_PERM = [1, 0, 2, 3]
WS = 32.0            # fp8 weight scale (ACT de-scales with 1/WS)

_CACHE = {}


def _build_program(t_steps):
    import concourse.mybir as mybir
    import concourse.tile as tile
    from concourse import bacc
    from concourse.masks import make_identity

    fp32 = mybir.dt.float32
    bf16 = mybir.dt.bfloat16
    fp8 = mybir.dt.float8e4
    Act = mybir.ActivationFunctionType
    Alu = mybir.AluOpType
    DR = mybir.MatmulPerfMode.DoubleRow

    ntb = t_steps * BL
    chunk = min(512, ntb)
    nch = ntb // chunk
    spc = chunk // BL   # steps per chunk
    lead = min(2, nch)  # xg chunks computed ahead

    nc = bacc.Bacc("TRN2", target_bir_lowering=False, debug=False)
    xT_d = nc.dram_tensor("xT", [128, KT, ntb], bf16, kind="ExternalInput")
    wih_d = nc.dram_tensor("wihT", [128, KT, 4 * H], bf16, kind="ExternalInput")
    whh_d = nc.dram_tensor("whhT", [128, KT, 4 * H], fp8, kind="ExternalInput")
    fcw_d = nc.dram_tensor("fcwT", [128, KT, NOUT], bf16, kind="ExternalInput")
    bias_d = nc.dram_tensor("bias", [128, MT], fp32, kind="ExternalInput")
    outT_d = nc.dram_tensor("outT", [NOUT // 128, 128, ntb], fp32, kind="ExternalOutput")

    gw = KT * BL  # 64 columns per gate group

    with tile.TileContext(nc) as tc:
        with (
            tc.tile_pool(name="weights", bufs=1) as wp,
            tc.tile_pool(name="state", bufs=1) as sp,
            tc.tile_pool(name="ring", bufs=lead + 1) as rp,
            tc.tile_pool(name="stage", bufs=3) as stp,
            tc.tile_pool(name="work", bufs=2) as wk,
            tc.tile_pool(name="cpool", bufs=2) as cp,
            tc.tile_pool(name="psg", bufs=2, space="PSUM") as psg,
            tc.tile_pool(name="psb", bufs=2, space="PSUM") as psb,
        ):
            xT = wp.tile([128, KT, ntb], bf16)
            wih = wp.tile([128, KT, 4 * H], bf16)
            whh = wp.tile([128, KT, 4 * H], fp8)
            fcw = wp.tile([128, KT, NOUT], bf16)
            bias = wp.tile([128, MT], fp32)
            ident = wp.tile([128, 128], bf16)
            # fp8 copy feeds the recurrence matmuls, bf16 copy feeds the FC
            h8 = sp.tile([128, KT, (t_steps + 1) * BL], fp8)
            hb = sp.tile([128, KT, (t_steps + 1) * BL], bf16)

            for ch in range(nch):
                nc.sync.dma_start(xT[:, :, ch * chunk:(ch + 1) * chunk],
                                  xT_d[:, :, ch * chunk:(ch + 1) * chunk])
            nc.sync.dma_start(wih[:], wih_d[:])
            nc.sync.dma_start(whh[:], whh_d[:])
            nc.sync.dma_start(fcw[:], fcw_d[:])
            nc.sync.dma_start(bias[:], bias_d[:])
            make_identity(nc, ident[:])
            nc.vector.memset(h8[:, :, 0:BL], 0.0)

            rings = {}
            xg_ps = {}
            fc_ps = [None]

            def get_ring(ch):
                if ch not in rings:
                    rings[ch] = rp.tile([128, MT, chunk], bf16, tag="ring",
                                        name=f"ring{ch}")
                return rings[ch]

            def xg_mm(ch, m, k):
                """One k-MM of the xg unit (ch, m); evacuates on k==KT-1."""
                ring = get_ring(ch)
                if k == 0:
                    xg_ps[(ch, m)] = psb.tile([128, chunk], fp32, tag="big",
                                              name=f"xgps{ch}_{m}")
                ps = xg_ps[(ch, m)]
                nc.tensor.matmul(
                    ps[:], wih[:, k, m * 128:(m + 1) * 128],
                    xT[:, k, ch * chunk:(ch + 1) * chunk],
                    start=(k == 0), stop=(k == KT - 1))
                if k == KT - 1:
                    nc.vector.tensor_scalar_add(ring[:, m, :], ps[:],
                                                bias[:, m:m + 1])
                    del xg_ps[(ch, m)]

            def fc_mm(ch, m, k):
                if k == 0:
                    fc_ps[0] = psb.tile([128, chunk], fp32, tag="big",
                                        name=f"fcps{m}_{ch}")
                ps = fc_ps[0]
                nc.tensor.matmul(
                    ps[:], fcw[:, k, m * 128:(m + 1) * 128],
                    hb[:, k, BL + ch * chunk:BL + (ch + 1) * chunk],
                    start=(k == 0), stop=(k == KT - 1))
                if k == KT - 1:
                    st = stp.tile([128, chunk], fp32, tag="ost")
                    nc.vector.tensor_copy(st[:], ps[:])
                    nc.sync.dma_start(
                        outT_d[m, :, ch * chunk:(ch + 1) * chunk], st[:])

            # prologue: first `lead` xg chunks
            for ch in range(lead):
                for m in range(MT):
                    for k in range(KT):
                        xg_mm(ch, m, k)

            n_fc_mm = (NOUT // 128) * nch * KT
            fc_done = 0
            xg_done = 0  # MMs emitted for chunks >= lead
            c_prev = None
            for t in range(t_steps):
                s = t % spc
                ch = t // spc
                ring = get_ring(ch)

                a = wk.tile([128, 4 * gw], fp32, tag="a")
                # psum banks: f | i,g (merged for one sigmoid) | o
                pf = psg.tile([128, gw], fp32, tag="pf", name="pf")
                pig = psg.tile([128, 2 * gw], fp32, tag="pig", name="pig")
                po = psg.tile([128, gw], fp32, tag="po", name="po")

                def gate_mms(ps, mlo, mhi, ring_cols):
                    # identity matmul seeds the bank with 32*xg from the ring
                    nc.tensor.matmul(ps[:], ident[:], ring_cols,
                                     start=True, stop=False)
                    for m in range(mlo, mhi):
                        for kp in range(KT // 2):
                            nc.tensor.matmul(
                                ps[:, (m - mlo) * BL:(m - mlo + 1) * BL],
                                whh[:, 2 * kp:2 * kp + 2, m * 128:(m + 1) * 128],
                                h8[:, 2 * kp:2 * kp + 2, t * BL:(t + 1) * BL],
                                start=False,
                                stop=(m == mhi - 1 and kp == KT // 2 - 1),
                                perf_mode=DR)

                gate_mms(pf, 0, 4, ring[:, 0:4, s * BL:(s + 1) * BL])
                gate_mms(pig, 4, 12, ring[:, 4:12, s * BL:(s + 1) * BL])
                gate_mms(po, 12, 16, ring[:, 12:16, s * BL:(s + 1) * BL])

                # ACT: sigmoid everything (scale de-applies the x32 weights)
                nc.scalar.activation(a[:, 0:gw], pf[:], Act.Sigmoid,
                                     scale=1.0 / WS)
                nc.scalar.activation(a[:, gw:3 * gw], pig[:], Act.Sigmoid,
                                     scale=1.0 / WS)
                nc.scalar.activation(a[:, 3 * gw:4 * gw], po[:], Act.Sigmoid,
                                     scale=1.0 / WS)

                # DVE spine: c = 2*(Gg-0.5)*Gi + Gf*c_prev ; h' = (u-0.5)*Go
                if t > 0:
                    c1 = wk.tile([128, gw], fp32, tag="c1")
                    nc.vector.tensor_mul(c1[:], a[:, 0:gw], c_prev[:])
                t1h = wk.tile([128, gw], fp32, tag="t1h")
                nc.vector.scalar_tensor_tensor(
                    t1h[:], a[:, 2 * gw:3 * gw], -0.5, a[:, gw:2 * gw],
                    Alu.add, Alu.mult)
                c_new = cp.tile([128, gw], fp32, tag="c")
                if t == 0:
                    nc.vector.tensor_scalar_mul(c_new[:], t1h[:], 2.0)
                else:
                    nc.vector.scalar_tensor_tensor(
                        c_new[:], t1h[:], 2.0, c1[:], Alu.mult, Alu.add)
                u = wk.tile([128, gw], fp32, tag="u")
                nc.scalar.activation(u[:], c_new[:], Act.Sigmoid, scale=2.0)
                u_r = u[:].rearrange("p (k b) -> p k b", b=BL)
                o_r = a[:, 3 * gw:4 * gw].rearrange("p (k b) -> p k b", b=BL)
                nc.vector.scalar_tensor_tensor(
                    h8[:, :, (t + 1) * BL:(t + 2) * BL], u_r, -0.5, o_r,
                    Alu.add, Alu.mult)
                nc.vector.scalar_tensor_tensor(
                    hb[:, :, (t + 1) * BL:(t + 2) * BL], u_r, -0.5, o_r,
                    Alu.add, Alu.mult)
                c_prev = c_new

                # stuffers AFTER the gate MMs: they run inside the spine
                # window on the in-order PE instead of delaying the step
                if ch + lead < nch:
                    tgt = 4 * MT * ch + (s + 1) * 4 * MT // spc
                    while xg_done < tgt:
                        u_i = xg_done % (4 * MT)
                        xg_mm(ch + lead, u_i // KT, u_i % KT)
                        xg_done += 1
                if t >= spc:
                    tgt = min(n_fc_mm, 4 * KT * (t // spc),
                              ((t - spc) * 4) // 7 + 1)
                    while fc_done < tgt:
                        u_i = fc_done
                        fc_mm(u_i // (KT * (NOUT // 128)),
                              (u_i // KT) % (NOUT // 128), u_i % KT)
                        fc_done += 1

                if ch - 1 in rings and s == spc - 1:
                    del rings[ch - 1]

            while fc_done < n_fc_mm:  # FC epilogue
                u_i = fc_done
                fc_mm(u_i // (KT * (NOUT // 128)), (u_i // KT) % (NOUT // 128),
                      u_i % KT)
                fc_done += 1

    nc.compile()
    return nc


def _get_program(t_steps=T):
    if t_steps not in _CACHE:
        _CACHE[t_steps] = _build_program(t_steps)
    return _CACHE[t_steps]


def _to_bf16(arr):
    import ml_dtypes

    return np.asarray(arr).astype(ml_dtypes.bfloat16)


def _to_fp8(arr):
    import ml_dtypes

    return np.asarray(arr).astype(ml_dtypes.float8_e4m3fn)


def _prep_weight_T(w_gate_rows, conv):
    """[rows, 512] (gate-permuted rows) -> lhsT layout [128, KT, rows]."""
    wt = np.ascontiguousarray(np.asarray(w_gate_rows, np.float32).T)
    return conv(wt.reshape(KT, 128, wt.shape[1]).transpose(1, 0, 2))


def _gate_perm_rows(w):
    blocks = np.split(np.asarray(w, np.float32), 4, axis=0)
    return np.concatenate([blocks[i] for i in GATE_PERM], axis=0)


def _g_row_scale(rows_scaled):
    """Scale the g-gate block (3rd group in [f,i,g,o] order) by 2."""
    out = rows_scaled.copy()
    out[2 * H:3 * H] *= 2.0
    return out


def _make_in_maps(x, w_ih_f, w_hh_f, b_ih_f, b_hh_f, w_ih_b, w_hh_b, b_ih_b,
                  b_hh_b, fc_w, fc_b, t_steps):
    per_dir = []
    for d, (wih, whh, bih, bhh) in enumerate(
        [(w_ih_f, w_hh_f, b_ih_f, b_hh_f), (w_ih_b, w_hh_b, b_ih_b, b_hh_b)]
    ):
        # [f,i,g,o] rows; xg path x WS (g-rows x2 more); recurrent weights
        # additionally x2 (stored state is h/2) -> x(2*WS), fp8
        wih_r = _g_row_scale(_gate_perm_rows(wih) * WS)
        whh_r = _g_row_scale(_gate_perm_rows(whh) * (2.0 * WS))
        bias_r = _g_row_scale(
            _gate_perm_rows(
                (np.asarray(bih) + np.asarray(bhh))[:, None]) * WS)[:, 0]
        per_dir.append({
            "wihT": _prep_weight_T(wih_r, _to_bf16),
            "whhT": _prep_weight_T(whh_r, _to_fp8),
            "fcwT": _prep_weight_T(np.ascontiguousarray(
                np.asarray(fc_w, np.float32)[:, d * H:(d + 1) * H]) * 2.0,
                _to_bf16),
            "bias": np.ascontiguousarray(
                bias_r.reshape(MT, 128).T).astype(np.float32),
        })
    in_maps = []
    for c in range(8):
        d, q = c // 4, c % 4
        xq = np.asarray(x)[:t_steps, q * BL:(q + 1) * BL, :]
        if d == 1:
            xq = xq[::-1]
        xT = xq.transpose(2, 0, 1).reshape(KT, 128, t_steps * BL).transpose(1, 0, 2)
        m = dict(per_dir[d])
        m["xT"] = _to_bf16(xT)
        in_maps.append(m)
    return in_maps


def _assemble(results, fc_b, t_steps):
    out = np.zeros((t_steps, B, NOUT), np.float32)
    for c in range(8):
        d, q = c // 4, c % 4
        oT = np.asarray(results[c]["outT"]).reshape(NOUT, t_steps, BL)
        part = oT.transpose(1, 2, 0)  # [t, b, out]
        if d == 1:
            part = part[::-1]
        out[:, q * BL:(q + 1) * BL, :] += part
    out += np.asarray(fc_b, np.float32)
    return out


def kernel(x, w_ih_f, w_hh_f, b_ih_f, b_hh_f, w_ih_b, w_hh_b, b_ih_b, b_hh_b,
           fc_w, fc_b, _t_steps=T, _trace=False, _trace_kwargs=None):
    from concourse.bass_utils import run_bass_kernel_spmd

    nc = _get_program(_t_steps)
    in_maps = _make_in_maps(x, w_ih_f, w_hh_f, b_ih_f, b_hh_f, w_ih_b, w_hh_b,
                            b_ih_b, b_hh_b, fc_w, fc_b, _t_steps)
    res = run_bass_kernel_spmd(
        nc, in_maps, core_ids=list(range(8)), trace=_trace,
        **(_trace_kwargs or {}),
    )
    out = _assemble(res.results, fc_b, _t_steps)
    if _trace:
        kernel._last_result = res
    return out


# revision 4
# speedup vs baseline: 1.7032x; 1.7032x over previous
"""BiLSTM (T=256, B=64, NIN=H=NOUT=512) Trainium2 kernel over 8 NeuronCores.

Sharding: direction (2) x batch-quarter (4) = 8 cores, SPMD (one program).
Each core runs one direction's LSTM for 16 batch rows (backward cores get
time-reversed x), then computes its half of the final FC:
    out = h_f @ fc_w[:, :H].T + h_b @ fc_w[:, H:].T + fc_b
The host sums the two partial FC outputs per batch quarter. No collectives.

V2 changes vs baseline:
  - Recurrence matmuls in fp8(e4m3) with DoubleRow perf mode: contraction
    256 per instruction -> 32 gate MMs/step instead of 64. Weights scaled
    x32 (avoids e4m3 subnormals); the sigmoid ACT op applies scale=1/32.
  - Sigmoid-only cell math (no tanh table):
      * g-gate rows pre-scaled x2 so sigma(2 zg) = (tanh(zg)+1)/2
      * stored state h' = h/2 with W_hh / fc_w pre-scaled x2
      * t1h = (Gg-0.5)*Gi, c = 2*t1h + Gf*c_prev, u = sigma(2c),
        h' = (u-0.5)*Go  -- each one fused scalar_tensor_tensor DVE op.
  - PSUM layout [f][i|g][o]: i,g share one bank so sigma(i,g) is a single
    merged ACT op (the spine's first activation).
  - h' written twice: fp8 copy for the recurrence rhs, bf16 copy for FC
    (fp8 h into FC fails the accuracy budget; bf16 copy passes at 7e-3).
  - xg/FC stuffer matmuls emitted AFTER each step's gate MMs so they fill
    the ACT/DVE spine window instead of delaying the gate matmuls.
"""

import numpy as np

T, B, NIN, H, NOUT = 256, 64, 512, 512, 512
BL = B // 4          # local batch per core (batch quarter)
KT = H // 128        # 4 k-tiles over the hidden/contraction dim
MT = (4 * H) // 128  # 16 m-tiles over the gate dim
# PyTorch gate blocks [i,f,g,o] -> our order [f,i,g,o]
GATE_PERM = [1, 0, 2, 3]
WS = 32.0            # fp8 weight scale (ACT de-scales with 1/WS)

_CACHE = {}


def _build_program(t_steps):
    import concourse.mybir as mybir
    import concourse.tile as tile
    from concourse import bacc
    from concourse.masks import make_identity

    fp32 = mybir.dt.float32
    bf16 = mybir.dt.bfloat16
    fp8 = mybir.dt.float8e4
    Act = mybir.ActivationFunctionType
    Alu = mybir.AluOpType
    DR = mybir.MatmulPerfMode.DoubleRow

    ntb = t_steps * BL
    chunk = min(512, ntb)
    nch = ntb // chunk
    spc = chunk // BL   # steps per chunk
    lead = min(2, nch)  # xg chunks computed ahead

    nc = bacc.Bacc("TRN2", target_bir_lowering=False, debug=False)
    xT_d = nc.dram_tensor("xT", [128, KT, ntb], bf16, kind="ExternalInput")
    wih_d = nc.dram_tensor("wihT", [128, KT, 4 * H], bf16, kind="ExternalInput")
    whh_d = nc.dram_tensor("whhT", [128, KT, 4 * H], fp8, kind="ExternalInput")
    fcw_d = nc.dram_tensor("fcwT", [128, KT, NOUT], bf16, kind="ExternalInput")
    bias_d = nc.dram_tensor("bias", [128, MT], fp32, kind="ExternalInput")
    outT_d = nc.dram_tensor("outT", [NOUT // 128, 128, ntb], fp32, kind="ExternalOutput")

    gw = KT * BL  # 64 columns per gate group

    with tile.TileContext(nc) as tc:
        with (
            tc.tile_pool(name="weights", bufs=1) as wp,
            tc.tile_pool(name="state", bufs=1) as sp,
            tc.tile_pool(name="ring", bufs=lead + 1) as rp,
            tc.tile_pool(name="stage", bufs=3) as stp,
            tc.tile_pool(name="work", bufs=2) as wk,
            tc.tile_pool(name="cpool", bufs=2) as cp,
            tc.tile_pool(name="psg", bufs=2, space="PSUM") as psg,
            tc.tile_pool(name="psb", bufs=2, space="PSUM") as psb,
        ):
            xT = wp.tile([128, KT, ntb], bf16)
            wih = wp.tile([128, KT, 4 * H], bf16)
            whh = wp.tile([128, KT, 4 * H], fp8)
            fcw = wp.tile([128, KT, NOUT], bf16)
            bias = wp.tile([128, MT], fp32)
            ident = wp.tile([128, 128], bf16)
            # fp8 copy feeds the recurrence matmuls, bf16 copy feeds the FC
            h8 = sp.tile([128, KT, (t_steps + 1) * BL], fp8)
            hb = sp.tile([128, KT, (t_steps + 1) * BL], bf16)

            for ch in range(nch):
                nc.sync.dma_start(xT[:, :, ch * chunk:(ch + 1) * chunk],
                                  xT_d[:, :, ch * chunk:(ch + 1) * chunk])
            nc.sync.dma_start(wih[:], wih_d[:])
            nc.sync.dma_start(whh[:], whh_d[:])
            nc.sync.dma_start(fcw[:], fcw_d[:])
            nc.sync.dma_start(bias[:], bias_d[:])
            make_identity(nc, ident[:])
            nc.vector.memset(h8[:, :, 0:BL], 0.0)

            rings = {}
            xg_ps = {}
            fc_ps = [None]

            def get_ring(ch):
                if ch not in rings:
                    rings[ch] = rp.tile([128, MT, chunk], bf16, tag="ring",
                                        name=f"ring{ch}")
                return rings[ch]

            def xg_mm(ch, m, k):
                """One k-MM of the xg unit (ch, m); evacuates on k==KT-1."""
                ring = get_ring(ch)
                if k == 0:
                    xg_ps[(ch, m)] = psb.tile([128, chunk], fp32, tag="big",
                                              name=f"xgps{ch}_{m}")
                ps = xg_ps[(ch, m)]
                nc.tensor.matmul(
                    ps[:], wih[:, k, m * 128:(m + 1) * 128],
                    xT[:, k, ch * chunk:(ch + 1) * chunk],
                    start=(k == 0), stop=(k == KT - 1))
                if k == KT - 1:
                    nc.vector.tensor_scalar_add(ring[:, m, :], ps[:],
                                                bias[:, m:m + 1])
                    del xg_ps[(ch, m)]

            def fc_mm(ch, m, k):
                if k == 0:
                    fc_ps[0] = psb.tile([128, chunk], fp32, tag="big",
                                        name=f"fcps{m}_{ch}")
                ps = fc_ps[0]
                nc.tensor.matmul(
                    ps[:], fcw[:, k, m * 128:(m + 1) * 128],
                    hb[:, k, BL + ch * chunk:BL + (ch + 1) * chunk],
                    start=(k == 0), stop=(k == KT - 1))
                if k == KT - 1:
                    st = stp.tile([128, chunk], fp32, tag="ost")
                    nc.vector.tensor_copy(st[:], ps[:])
                    nc.sync.dma_start(
                        outT_d[m, :, ch * chunk:(ch + 1) * chunk], st[:])

            # prologue: first `lead` xg chunks
            for ch in range(lead):
                for m in range(MT):
                    for k in range(KT):
                        xg_mm(ch, m, k)

            n_fc_mm = (NOUT // 128) * nch * KT
            fc_done = 0
            xg_done = 0  # MMs emitted for chunks >= lead
            c_prev = None
            for t in range(t_steps):
                s = t % spc
                ch = t // spc
                ring = get_ring(ch)

                a = wk.tile([128, 4 * gw], fp32, tag="a")
                # psum banks: f | i,g (merged for one sigmoid) | o
                pf = psg.tile([128, gw], fp32, tag="pf", name="pf")
                pig = psg.tile([128, 2 * gw], fp32, tag="pig", name="pig")
                po = psg.tile([128, gw], fp32, tag="po", name="po")

                def gate_mms(ps, mlo, mhi, ring_cols):
                    # identity matmul seeds the bank with 32*xg from the ring
                    nc.tensor.matmul(ps[:], ident[:], ring_cols,
                                     start=True, stop=False)
                    for m in range(mlo, mhi):
                        for k in range(KT):
                            nc.tensor.matmul(
                                ps[:, (m - mlo) * BL:(m - mlo + 1) * BL],
                                whh[:, k, m * 128:(m + 1) * 128],
                                h8[:, k, t * BL:(t + 1) * BL],
                                start=False,
                                stop=(m == mhi - 1 and k == KT - 1))

                gate_mms(pf, 0, 4, ring[:, 0:4, s * BL:(s + 1) * BL])
                gate_mms(pig, 4, 12, ring[:, 4:12, s * BL:(s + 1) * BL])
                gate_mms(po, 12, 16, ring[:, 12:16, s * BL:(s + 1) * BL])

                # ACT: sigmoid everything (scale de-applies the x32 weights)
                nc.scalar.activation(a[:, 0:gw], pf[:], Act.Sigmoid,
                                     scale=1.0 / WS)
                nc.scalar.activation(a[:, gw:3 * gw], pig[:], Act.Sigmoid,
                                     scale=1.0 / WS)
                nc.scalar.activation(a[:, 3 * gw:4 * gw], po[:], Act.Sigmoid,
                                     scale=1.0 / WS)

                # DVE spine: c = 2*(Gg-0.5)*Gi + Gf*c_prev ; h' = (u-0.5)*Go
                if t > 0:
                    c1 = wk.tile([128, gw], fp32, tag="c1")
                    nc.vector.tensor_mul(c1[:], a[:, 0:gw], c_prev[:])
                t1h = wk.tile([128, gw], fp32, tag="t1h")
                nc.vector.scalar_tensor_tensor(
                    t1h[:], a[:, 2 * gw:3 * gw], -0.5, a[:, gw:2 * gw],
                    Alu.add, Alu.mult)
                c_new = cp.tile([128, gw], fp32, tag="c")
                if t == 0:
                    nc.vector.tensor_scalar_mul(c_new[:], t1h[:], 2.0)
                else:
                    nc.vector.scalar_tensor_tensor(
                        c_new[:], t1h[:], 2.0, c1[:], Alu.mult, Alu.add)
                u = wk.tile([128, gw], fp32, tag="u")
                nc.scalar.activation(u[:], c_new[:], Act.Sigmoid, scale=2.0)
                u_r = u[:].rearrange("p (k b) -> p k b", b=BL)
                o_r = a[:, 3 * gw:4 * gw].rearrange("p (k b) -> p k b", b=BL)
                nc.vector.scalar_tensor_tensor(
                    h8[:, :, (t + 1) * BL:(t + 2) * BL], u_r, -0.5, o_r,
                    Alu.add, Alu.mult)
                nc.vector.scalar_tensor_tensor(
                    hb[:, :, (t + 1) * BL:(t + 2) * BL], u_r, -0.5, o_r,
                    Alu.add, Alu.mult)
                c_prev = c_new

                # stuffers AFTER the gate MMs: they run inside the spine
                # window on the in-order PE instead of delaying the step
                if ch + lead < nch:
                    tgt = 4 * MT * ch + (s + 1) * 4 * MT // spc
                    while xg_done < tgt:
                        u_i = xg_done % (4 * MT)
                        xg_mm(ch + lead, u_i // KT, u_i % KT)
                        xg_done += 1
                if t >= spc:
                    tgt = min(n_fc_mm, 4 * KT * (t // spc),
                              ((t - spc) * 4) // 7 + 1)
                    while fc_done < tgt:
                        u_i = fc_done
                        fc_mm(u_i // (KT * (NOUT // 128)),
                              (u_i // KT) % (NOUT // 128), u_i % KT)
                        fc_done += 1

                if ch - 1 in rings and s == spc - 1:
                    del rings[ch - 1]

            while fc_done < n_fc_mm:  # FC epilogue
                u_i = fc_done
                fc_mm(u_i // (KT * (NOUT // 128)), (u_i // KT) % (NOUT // 128),
                      u_i % KT)
                fc_done += 1

    nc.compile()
    return nc


def _get_program(t_steps=T):
    if t_steps not in _CACHE:
        _CACHE[t_steps] = _build_program(t_steps)
    return _CACHE[t_steps]


def _to_bf16(arr):
    import ml_dtypes

    return np.asarray(arr).astype(ml_dtypes.bfloat16)


def _to_fp8(arr):
    import ml_dtypes

    return np.asarray(arr).astype(ml_dtypes.float8_e4m3fn)


def _prep_weight_T(w_gate_rows, conv):
    """[rows, 512] (gate-permuted rows) -> lhsT layout [128, KT, rows]."""
    wt = np.ascontiguousarray(np.asarray(w_gate_rows, np.float32).T)
    return conv(wt.reshape(KT, 128, wt.shape[1]).transpose(1, 0, 2))


def _gate_perm_rows(w):
    blocks = np.split(np.asarray(w, np.float32), 4, axis=0)
    return np.concatenate([blocks[i] for i in GATE_PERM], axis=0)


def _g_row_scale(rows_scaled):
    """Scale the g-gate block (3rd group in [f,i,g,o] order) by 2."""
    out = rows_scaled.copy()
    out[2 * H:3 * H] *= 2.0
    return out


def _make_in_maps(x, w_ih_f, w_hh_f, b_ih_f, b_hh_f, w_ih_b, w_hh_b, b_ih_b,
                  b_hh_b, fc_w, fc_b, t_steps):
    per_dir = []
    for d, (wih, whh, bih, bhh) in enumerate(
        [(w_ih_f, w_hh_f, b_ih_f, b_hh_f), (w_ih_b, w_hh_b, b_ih_b, b_hh_b)]
    ):
        # [f,i,g,o] rows; xg path x WS (g-rows x2 more); recurrent weights
        # additionally x2 (stored state is h/2) -> x(2*WS), fp8
        wih_r = _g_row_scale(_gate_perm_rows(wih) * WS)
        whh_r = _g_row_scale(_gate_perm_rows(whh) * (2.0 * WS))
        bias_r = _g_row_scale(
            _gate_perm_rows(
                (np.asarray(bih) + np.asarray(bhh))[:, None]) * WS)[:, 0]
        per_dir.append({
            "wihT": _prep_weight_T(wih_r, _to_bf16),
            "whhT": _prep_weight_T(whh_r, _to_fp8),
            "fcwT": _prep_weight_T(np.ascontiguousarray(
                np.asarray(fc_w, np.float32)[:, d * H:(d + 1) * H]) * 2.0,
                _to_bf16),
            "bias": np.ascontiguousarray(
                bias_r.reshape(MT, 128).T).astype(np.float32),
        })
    in_maps = []
    for c in range(8):
        d, q = c // 4, c % 4
        xq = np.asarray(x)[:t_steps, q * BL:(q + 1) * BL, :]
        if d == 1:
            xq = xq[::-1]
        xT = xq.transpose(2, 0, 1).reshape(KT, 128, t_steps * BL).transpose(1, 0, 2)
        m = dict(per_dir[d])
        m["xT"] = _to_bf16(xT)
        in_maps.append(m)
    return in_maps


def _assemble(results, fc_b, t_steps):
    out = np.zeros((t_steps, B, NOUT), np.float32)
    for c in range(8):
        d, q = c // 4, c % 4
        oT = np.asarray(results[c]["outT"]).reshape(NOUT, t_steps, BL)
        part = oT.transpose(1, 2, 0)  # [t, b, out]
        if d == 1:
            part = part[::-1]
        out[:, q * BL:(q + 1) * BL, :] += part
    out += np.asarray(fc_b, np.float32)
    return out


def kernel(x, w_ih_f, w_hh_f, b_ih_f, b_hh_f, w_ih_b, w_hh_b, b_ih_b, b_hh_b,
           fc_w, fc_b, _t_steps=T, _trace=False, _trace_kwargs=None):
    from concourse.bass_utils import run_bass_kernel_spmd

    nc = _get_program(_t_steps)
    in_maps = _make_in_maps(x, w_ih_f, w_hh_f, b_ih_f, b_hh_f, w_ih_b, w_hh_b,
                            b_ih_b, b_hh_b, fc_w, fc_b, _t_steps)
    res = run_bass_kernel_spmd(
        nc, in_maps, core_ids=list(range(8)), trace=_trace,
        **(_trace_kwargs or {}),
    )
    out = _assemble(res.results, fc_b, _t_steps)
    if _trace:
        kernel._last_result = res
    return out


# revision 7
# speedup vs baseline: 1.7345x; 1.0183x over previous
"""BiLSTM (T=256, B=64, NIN=H=NOUT=512) Trainium2 kernel over 8 NeuronCores.

Sharding: direction (2) x batch-quarter (4) = 8 cores, SPMD (one program).
Each core runs one direction's LSTM for 16 batch rows (backward cores get
time-reversed x), then computes its half of the final FC:
    out = h_f @ fc_w[:, :H].T + h_b @ fc_w[:, H:].T + fc_b
The host sums the two partial FC outputs per batch quarter. No collectives.

V2 changes vs baseline:
  - Recurrence matmuls in fp8(e4m3) with DoubleRow perf mode: contraction
    256 per instruction -> 32 gate MMs/step instead of 64. Weights scaled
    x32 (avoids e4m3 subnormals); the sigmoid ACT op applies scale=1/32.
  - Sigmoid-only cell math (no tanh table):
      * g-gate rows pre-scaled x2 so sigma(2 zg) = (tanh(zg)+1)/2
      * stored state h' = h/2 with W_hh / fc_w pre-scaled x2
      * t1h = (Gg-0.5)*Gi, c = 2*t1h + Gf*c_prev, u = sigma(2c),
        h' = (u-0.5)*Go  -- each one fused scalar_tensor_tensor DVE op.
  - PSUM layout [f][i|g][o]: i,g share one bank so sigma(i,g) is a single
    merged ACT op (the spine's first activation).
  - h' written twice: fp8 copy for the recurrence rhs, bf16 copy for FC
    (fp8 h into FC fails the accuracy budget; bf16 copy passes at 7e-3).
  - xg/FC stuffer matmuls emitted AFTER each step's gate MMs so they fill
    the ACT/DVE spine window instead of delaying the gate matmuls.
"""

import numpy as np

T, B, NIN, H, NOUT = 256, 64, 512, 512, 512
BL = B // 4          # local batch per core (batch quarter)
KT = H // 128        # 4 k-tiles over the hidden/contraction dim
MT = (4 * H) // 128  # 16 m-tiles over the gate dim
# PyTorch gate blocks [i,f,g,o] -> our order [f,i,g,o]
GATE_PERM = [1, 0, 2, 3]
WS = 32.0            # fp8 weight scale (ACT de-scales with 1/WS)

_CACHE = {}


def _build_program(t_steps):
    import concourse.mybir as mybir
    import concourse.tile as tile
    from concourse import bacc
    from concourse.masks import make_identity

    fp32 = mybir.dt.float32
    bf16 = mybir.dt.bfloat16
    fp8 = mybir.dt.float8e4
    Act = mybir.ActivationFunctionType
    Alu = mybir.AluOpType
    DR = mybir.MatmulPerfMode.DoubleRow

    ntb = t_steps * BL
    chunk = min(512, ntb)
    nch = ntb // chunk
    spc = chunk // BL   # steps per chunk
    lead = min(2, nch)  # xg chunks computed ahead

    nc = bacc.Bacc("TRN2", target_bir_lowering=False, debug=False)
    xT_d = nc.dram_tensor("xT", [128, KT, ntb], bf16, kind="ExternalInput")
    wih_d = nc.dram_tensor("wihT", [128, KT, 4 * H], bf16, kind="ExternalInput")
    whh_d = nc.dram_tensor("whhT", [128, KT, 4 * H], fp8, kind="ExternalInput")
    fcw_d = nc.dram_tensor("fcwT", [128, KT, NOUT], bf16, kind="ExternalInput")
    bias_d = nc.dram_tensor("bias", [128, MT], fp32, kind="ExternalInput")
    outT_d = nc.dram_tensor("outT", [NOUT // 128, 128, ntb], fp32, kind="ExternalOutput")

    gw = KT * BL  # 64 columns per gate group

    with tile.TileContext(nc) as tc:
        with (
            tc.tile_pool(name="weights", bufs=1) as wp,
            tc.tile_pool(name="state", bufs=1) as sp,
            tc.tile_pool(name="ring", bufs=lead + 1) as rp,
            tc.tile_pool(name="stage", bufs=3) as stp,
            tc.tile_pool(name="work", bufs=2) as wk,
            tc.tile_pool(name="cpool", bufs=2) as cp,
            tc.tile_pool(name="psg", bufs=2, space="PSUM") as psg,
            tc.tile_pool(name="psb", bufs=2, space="PSUM") as psb,
        ):
            xT = wp.tile([128, KT, ntb], bf16)
            wih = wp.tile([128, KT, 4 * H], bf16)
            whh = wp.tile([128, KT, 4 * H], fp8)
            fcw = wp.tile([128, KT, NOUT], bf16)
            bias = wp.tile([128, MT], fp32)
            ident = wp.tile([128, 128], bf16)
            # fp8 copy feeds the recurrence matmuls, bf16 copy feeds the FC
            h8 = sp.tile([128, KT, (t_steps + 1) * BL], fp8)
            hb = sp.tile([128, KT, (t_steps + 1) * BL], bf16)

            for ch in range(nch):
                nc.sync.dma_start(xT[:, :, ch * chunk:(ch + 1) * chunk],
                                  xT_d[:, :, ch * chunk:(ch + 1) * chunk])
            nc.sync.dma_start(wih[:], wih_d[:])
            nc.sync.dma_start(whh[:], whh_d[:])
            nc.sync.dma_start(fcw[:], fcw_d[:])
            nc.sync.dma_start(bias[:], bias_d[:])
            make_identity(nc, ident[:])
            nc.vector.memset(h8[:, :, 0:BL], 0.0)

            rings = {}
            xg_ps = {}
            fc_ps = [None]

            def get_ring(ch):
                if ch not in rings:
                    rings[ch] = rp.tile([128, MT, chunk], bf16, tag="ring",
                                        name=f"ring{ch}")
                return rings[ch]

            def xg_mm(ch, m, k):
                """One k-MM of the xg unit (ch, m); evacuates on k==KT-1."""
                ring = get_ring(ch)
                if k == 0:
                    xg_ps[(ch, m)] = psb.tile([128, chunk], fp32, tag="big",
                                              name=f"xgps{ch}_{m}")
                ps = xg_ps[(ch, m)]
                nc.tensor.matmul(
                    ps[:], wih[:, k, m * 128:(m + 1) * 128],
                    xT[:, k, ch * chunk:(ch + 1) * chunk],
                    start=(k == 0), stop=(k == KT - 1))
                if k == KT - 1:
                    nc.vector.tensor_scalar_add(ring[:, m, :], ps[:],
                                                bias[:, m:m + 1])
                    del xg_ps[(ch, m)]

            def fc_mm(ch, m, k):
                if k == 0:
                    fc_ps[0] = psb.tile([128, chunk], fp32, tag="big",
                                        name=f"fcps{m}_{ch}")
                ps = fc_ps[0]
                nc.tensor.matmul(
                    ps[:], fcw[:, k, m * 128:(m + 1) * 128],
                    hb[:, k, BL + ch * chunk:BL + (ch + 1) * chunk],
                    start=(k == 0), stop=(k == KT - 1))
                if k == KT - 1:
                    st = stp.tile([128, chunk], fp32, tag="ost")
                    nc.vector.tensor_copy(st[:], ps[:])
                    nc.sync.dma_start(
                        outT_d[m, :, ch * chunk:(ch + 1) * chunk], st[:])

            # prologue: first `lead` xg chunks
            for ch in range(lead):
                for m in range(MT):
                    for k in range(KT):
                        xg_mm(ch, m, k)

            n_fc_mm = (NOUT // 128) * nch * KT
            fc_done = 0
            xg_done = 0  # MMs emitted for chunks >= lead
            c_prev = None
            for t in range(t_steps):
                s = t % spc
                ch = t // spc
                ring = get_ring(ch)

                a = wk.tile([128, 4 * gw], fp32, tag="a")
                # psum banks: f | i,g (merged for one sigmoid) | o
                pf = psg.tile([128, gw], fp32, tag="pf", name="pf")
                pig = psg.tile([128, 2 * gw], fp32, tag="pig", name="pig")
                po = psg.tile([128, gw], fp32, tag="po", name="po")

                def gate_mms(ps, mlo, mhi):
                    for m in range(mlo, mhi):
                        for k in range(KT):
                            nc.tensor.matmul(
                                ps[:, (m - mlo) * BL:(m - mlo + 1) * BL],
                                whh[:, k, m * 128:(m + 1) * 128],
                                h8[:, k, t * BL:(t + 1) * BL],
                                start=False,
                                stop=(m == mhi - 1 and k == KT - 1))

                # i,g first: sigma(i,g) anchors the serial spine, so its
                # matmuls must finish as early as possible.  Seeds adjacent
                # (identical ident lhsT back-to-back: probes LDW dedup).
                nc.tensor.matmul(pig[:], ident[:],
                                 ring[:, 4:12, s * BL:(s + 1) * BL],
                                 start=True, stop=False)
                nc.tensor.matmul(pf[:], ident[:],
                                 ring[:, 0:4, s * BL:(s + 1) * BL],
                                 start=True, stop=False)
                nc.tensor.matmul(po[:], ident[:],
                                 ring[:, 12:16, s * BL:(s + 1) * BL],
                                 start=True, stop=False)
                gate_mms(pig, 4, 12)
                gate_mms(pf, 0, 4)
                gate_mms(po, 12, 16)

                # ACT order: sig(ig) -> sig(f) -> sig(o) -> sig(2c); sig(o)
                # fills the ACT gap between sig(f) and sig(2c)
                nc.scalar.activation(a[:, gw:3 * gw], pig[:], Act.Sigmoid,
                                     scale=1.0 / WS)
                nc.scalar.activation(a[:, 0:gw], pf[:], Act.Sigmoid,
                                     scale=1.0 / WS)
                nc.scalar.activation(a[:, 3 * gw:4 * gw], po[:], Act.Sigmoid,
                                     scale=1.0 / WS)

                # DVE spine: t1h first (its dep sig(ig) lands before sig(f))
                t1h = wk.tile([128, gw], fp32, tag="t1h")
                nc.vector.scalar_tensor_tensor(
                    t1h[:], a[:, 2 * gw:3 * gw], -0.5, a[:, gw:2 * gw],
                    Alu.add, Alu.mult)
                if t > 0:
                    c1 = wk.tile([128, gw], fp32, tag="c1")
                    nc.vector.tensor_mul(c1[:], a[:, 0:gw], c_prev[:])
                c_new = cp.tile([128, gw], fp32, tag="c")
                if t == 0:
                    nc.vector.tensor_scalar_mul(c_new[:], t1h[:], 2.0)
                else:
                    nc.vector.scalar_tensor_tensor(
                        c_new[:], t1h[:], 2.0, c1[:], Alu.mult, Alu.add)
                u = wk.tile([128, gw], fp32, tag="u")
                nc.scalar.activation(u[:], c_new[:], Act.Sigmoid, scale=2.0)
                u_r = u[:].rearrange("p (k b) -> p k b", b=BL)
                o_r = a[:, 3 * gw:4 * gw].rearrange("p (k b) -> p k b", b=BL)
                nc.vector.scalar_tensor_tensor(
                    h8[:, :, (t + 1) * BL:(t + 2) * BL], u_r, -0.5, o_r,
                    Alu.add, Alu.mult)
                nc.vector.scalar_tensor_tensor(
                    hb[:, :, (t + 1) * BL:(t + 2) * BL], u_r, -0.5, o_r,
                    Alu.add, Alu.mult)
                c_prev = c_new

                # stuffers AFTER the gate MMs: they run inside the spine
                # window on the in-order PE instead of delaying the step
                if ch + lead < nch:
                    tgt = 4 * MT * ch + (s + 1) * 4 * MT // spc
                    while xg_done < tgt:
                        u_i = xg_done % (4 * MT)
                        xg_mm(ch + lead, u_i // KT, u_i % KT)
                        xg_done += 1
                if t >= spc:
                    tgt = min(n_fc_mm, 4 * KT * (t // spc),
                              ((t - spc) * 4) // 7 + 1)
                    while fc_done < tgt:
                        u_i = fc_done
                        fc_mm(u_i // (KT * (NOUT // 128)),
                              (u_i // KT) % (NOUT // 128), u_i % KT)
                        fc_done += 1

                if ch - 1 in rings and s == spc - 1:
                    del rings[ch - 1]

            while fc_done < n_fc_mm:  # FC epilogue
                u_i = fc_done
                fc_mm(u_i // (KT * (NOUT // 128)), (u_i // KT) % (NOUT // 128),
                      u_i % KT)
                fc_done += 1

    nc.compile()
    return nc


def _get_program(t_steps=T):
    if t_steps not in _CACHE:
        _CACHE[t_steps] = _build_program(t_steps)
    return _CACHE[t_steps]


def _to_bf16(arr):
    import ml_dtypes

    return np.asarray(arr).astype(ml_dtypes.bfloat16)


def _to_fp8(arr):
    import ml_dtypes

    return np.asarray(arr).astype(ml_dtypes.float8_e4m3fn)


def _prep_weight_T(w_gate_rows, conv):
    """[rows, 512] (gate-permuted rows) -> lhsT layout [128, KT, rows]."""
    wt = np.ascontiguousarray(np.asarray(w_gate_rows, np.float32).T)
    return conv(wt.reshape(KT, 128, wt.shape[1]).transpose(1, 0, 2))


def _gate_perm_rows(w):
    blocks = np.split(np.asarray(w, np.float32), 4, axis=0)
    return np.concatenate([blocks[i] for i in GATE_PERM], axis=0)


def _g_row_scale(rows_scaled):
    """Scale the g-gate block (3rd group in [f,i,g,o] order) by 2."""
    out = rows_scaled.copy()
    out[2 * H:3 * H] *= 2.0
    return out


def _make_in_maps(x, w_ih_f, w_hh_f, b_ih_f, b_hh_f, w_ih_b, w_hh_b, b_ih_b,
                  b_hh_b, fc_w, fc_b, t_steps):
    per_dir = []
    for d, (wih, whh, bih, bhh) in enumerate(
        [(w_ih_f, w_hh_f, b_ih_f, b_hh_f), (w_ih_b, w_hh_b, b_ih_b, b_hh_b)]
    ):
        # [f,i,g,o] rows; xg path x WS (g-rows x2 more); recurrent weights
        # additionally x2 (stored state is h/2) -> x(2*WS), fp8
        wih_r = _g_row_scale(_gate_perm_rows(wih) * WS)
        whh_r = _g_row_scale(_gate_perm_rows(whh) * (2.0 * WS))
        bias_r = _g_row_scale(
            _gate_perm_rows(
                (np.asarray(bih) + np.asarray(bhh))[:, None]) * WS)[:, 0]
        per_dir.append({
            "wihT": _prep_weight_T(wih_r, _to_bf16),
            "whhT": _prep_weight_T(whh_r, _to_fp8),
            "fcwT": _prep_weight_T(np.ascontiguousarray(
                np.asarray(fc_w, np.float32)[:, d * H:(d + 1) * H]) * 2.0,
                _to_bf16),
            "bias": np.ascontiguousarray(
                bias_r.reshape(MT, 128).T).astype(np.float32),
        })
    in_maps = []
    for c in range(8):
        d, q = c // 4, c % 4
        xq = np.asarray(x)[:t_steps, q * BL:(q + 1) * BL, :]
        if d == 1:
            xq = xq[::-1]
        xT = xq.transpose(2, 0, 1).reshape(KT, 128, t_steps * BL).transpose(1, 0, 2)
        m = dict(per_dir[d])
        m["xT"] = _to_bf16(xT)
        in_maps.append(m)
    return in_maps


def _assemble(results, fc_b, t_steps):
    out = np.zeros((t_steps, B, NOUT), np.float32)
    for c in range(8):
        d, q = c // 4, c % 4
        oT = np.asarray(results[c]["outT"]).reshape(NOUT, t_steps, BL)
        part = oT.transpose(1, 2, 0)  # [t, b, out]
        if d == 1:
            part = part[::-1]
        out[:, q * BL:(q + 1) * BL, :] += part
    out += np.asarray(fc_b, np.float32)
    return out


def kernel(x, w_ih_f, w_hh_f, b_ih_f, b_hh_f, w_ih_b, w_hh_b, b_ih_b, b_hh_b,
           fc_w, fc_b, _t_steps=T, _trace=False, _trace_kwargs=None):
    from concourse.bass_utils import run_bass_kernel_spmd

    nc = _get_program(_t_steps)
    in_maps = _make_in_maps(x, w_ih_f, w_hh_f, b_ih_f, b_hh_f, w_ih_b, w_hh_b,
                            b_ih_b, b_hh_b, fc_w, fc_b, _t_steps)
    res = run_bass_kernel_spmd(
        nc, in_maps, core_ids=list(range(8)), trace=_trace,
        **(_trace_kwargs or {}),
    )
    out = _assemble(res.results, fc_b, _t_steps)
    if _trace:
        kernel._last_result = res
    return out


# revision 17
# speedup vs baseline: 1.7431x; 1.0050x over previous
"""BiLSTM (T=256, B=64, NIN=H=NOUT=512) Trainium2 kernel over 8 NeuronCores.

Sharding: direction (2) x batch-quarter (4) = 8 cores, SPMD (one program).
Each core runs one direction's LSTM for 16 batch rows (backward cores get
time-reversed x), then computes its half of the final FC:
    out = h_f @ fc_w[:, :H].T + h_b @ fc_w[:, H:].T + fc_b
The host sums the two partial FC outputs per batch quarter. No collectives.

V2 changes vs baseline:
  - Recurrence matmuls in fp8(e4m3) with DoubleRow perf mode: contraction
    256 per instruction -> 32 gate MMs/step instead of 64. Weights scaled
    x32 (avoids e4m3 subnormals); the sigmoid ACT op applies scale=1/32.
  - Sigmoid-only cell math (no tanh table):
      * g-gate rows pre-scaled x2 so sigma(2 zg) = (tanh(zg)+1)/2
      * stored state h' = h/2 with W_hh / fc_w pre-scaled x2
      * t1h = (Gg-0.5)*Gi, c = 2*t1h + Gf*c_prev, u = sigma(2c),
        h' = (u-0.5)*Go  -- each one fused scalar_tensor_tensor DVE op.
  - PSUM layout [f][i|g][o]: i,g share one bank so sigma(i,g) is a single
    merged ACT op (the spine's first activation).
  - h' written twice: fp8 copy for the recurrence rhs, bf16 copy for FC
    (fp8 h into FC fails the accuracy budget; bf16 copy passes at 7e-3).
  - xg/FC stuffer matmuls emitted AFTER each step's gate MMs so they fill
    the ACT/DVE spine window instead of delaying the gate matmuls.
"""

import numpy as np

T, B, NIN, H, NOUT = 256, 64, 512, 512, 512
BL = B // 4          # local batch per core (batch quarter)
KT = H // 128        # 4 k-tiles over the hidden/contraction dim
MT = (4 * H) // 128  # 16 m-tiles over the gate dim
# PyTorch gate blocks [i,f,g,o] -> our order [f,i,g,o]
GATE_PERM = [1, 0, 2, 3]
WS = 32.0            # fp8 weight scale (ACT de-scales with 1/WS)

_CACHE = {}


def _build_program(t_steps):
    import concourse.mybir as mybir
    import concourse.tile as tile
    from concourse import bacc
    from concourse.masks import make_identity

    fp32 = mybir.dt.float32
    bf16 = mybir.dt.bfloat16
    fp8 = mybir.dt.float8e4
    Act = mybir.ActivationFunctionType
    Alu = mybir.AluOpType
    DR = mybir.MatmulPerfMode.DoubleRow

    ntb = t_steps * BL
    chunk = min(512, ntb)
    nch = ntb // chunk
    spc = chunk // BL   # steps per chunk
    lead = min(1, nch)  # xg chunks computed ahead (chunk 0 only upfront)

    nc = bacc.Bacc("TRN2", target_bir_lowering=False, debug=False)
    xT_d = nc.dram_tensor("xT", [128, KT, ntb], bf16, kind="ExternalInput")
    wih_d = nc.dram_tensor("wihT", [128, KT, 4 * H], bf16, kind="ExternalInput")
    whh_d = nc.dram_tensor("whhT", [128, KT, 4 * H], fp8, kind="ExternalInput")
    fcw_d = nc.dram_tensor("fcwT", [128, KT, NOUT], bf16, kind="ExternalInput")
    bias_d = nc.dram_tensor("bias", [128, MT], fp32, kind="ExternalInput")
    outT_d = nc.dram_tensor("outT", [NOUT // 128, 128, ntb], fp32, kind="ExternalOutput")

    gw = KT * BL  # 64 columns per gate group

    with tile.TileContext(nc) as tc:
        with (
            tc.tile_pool(name="weights", bufs=1) as wp,
            tc.tile_pool(name="state", bufs=1) as sp,
            tc.tile_pool(name="ring", bufs=lead + 1) as rp,
            tc.tile_pool(name="stage", bufs=3) as stp,
            tc.tile_pool(name="work", bufs=2) as wk,
            tc.tile_pool(name="cpool", bufs=2) as cp,
            tc.tile_pool(name="psg", bufs=2, space="PSUM") as psg,
            tc.tile_pool(name="psb", bufs=2, space="PSUM") as psb,
        ):
            xT = wp.tile([128, KT, ntb], bf16)
            wih = wp.tile([128, KT, 4 * H], bf16)
            whh = wp.tile([128, KT, 4 * H], fp8)
            fcw = wp.tile([128, KT, NOUT], bf16)
            bias = wp.tile([128, MT], fp32)
            ident = wp.tile([128, 128], bf16)
            # fp8 copy feeds the recurrence matmuls, bf16 copy feeds the FC
            h8 = sp.tile([128, KT, (t_steps + 1) * BL], fp8)
            hb = sp.tile([128, KT, (t_steps + 1) * BL], bf16)

            for ch in range(nch):
                nc.sync.dma_start(xT[:, :, ch * chunk:(ch + 1) * chunk],
                                  xT_d[:, :, ch * chunk:(ch + 1) * chunk])
            nc.sync.dma_start(wih[:], wih_d[:])
            nc.sync.dma_start(whh[:], whh_d[:])
            nc.sync.dma_start(fcw[:], fcw_d[:])
            nc.sync.dma_start(bias[:], bias_d[:])
            make_identity(nc, ident[:])
            nc.vector.memset(h8[:, :, 0:BL], 0.0)

            rings = {}
            xg_ps = {}
            fc_ps = [None]

            def get_ring(ch):
                if ch not in rings:
                    rings[ch] = rp.tile([128, MT, chunk], bf16, tag="ring",
                                        name=f"ring{ch}")
                return rings[ch]

            def xg_mm(ch, m, k):
                """One k-MM of the xg unit (ch, m); evacuates on k==KT-1."""
                ring = get_ring(ch)
                if k == 0:
                    xg_ps[(ch, m)] = psb.tile([128, chunk], fp32, tag="big",
                                              name=f"xgps{ch}_{m}")
                ps = xg_ps[(ch, m)]
                nc.tensor.matmul(
                    ps[:], wih[:, k, m * 128:(m + 1) * 128],
                    xT[:, k, ch * chunk:(ch + 1) * chunk],
                    start=(k == 0), stop=(k == KT - 1))
                if k == KT - 1:
                    # two half-evacuations bound the damage if the list
                    # scheduler slots one into the DVE spine
                    hc = chunk // 2
                    nc.vector.tensor_scalar_add(ring[:, m, 0:hc], ps[:, 0:hc],
                                                bias[:, m:m + 1])
                    nc.vector.tensor_scalar_add(ring[:, m, hc:chunk],
                                                ps[:, hc:chunk],
                                                bias[:, m:m + 1])
                    del xg_ps[(ch, m)]

            def fc_mm(ch, m, k):
                if k == 0:
                    fc_ps[0] = psb.tile([128, chunk], fp32, tag="big",
                                        name=f"fcps{m}_{ch}")
                ps = fc_ps[0]
                nc.tensor.matmul(
                    ps[:], fcw[:, k, m * 128:(m + 1) * 128],
                    hb[:, k, BL + ch * chunk:BL + (ch + 1) * chunk],
                    start=(k == 0), stop=(k == KT - 1))
                if k == KT - 1:
                    st = stp.tile([128, chunk], fp32, tag="ost")
                    hc = chunk // 2
                    nc.vector.tensor_copy(st[:, 0:hc], ps[:, 0:hc])
                    nc.vector.tensor_copy(st[:, hc:chunk], ps[:, hc:chunk])
                    nc.sync.dma_start(
                        outT_d[m, :, ch * chunk:(ch + 1) * chunk], st[:])

            # prologue: first `lead` xg chunks
            for ch in range(lead):
                for m in range(MT):
                    for k in range(KT):
                        xg_mm(ch, m, k)

            n_fc_mm = (NOUT // 128) * nch * KT
            fc_done = 0
            xg_done = 0  # MMs emitted for chunks >= lead
            c_prev = None
            for t in range(t_steps):
                s = t % spc
                ch = t // spc
                ring = get_ring(ch)

                a = wk.tile([128, 4 * gw], fp32, tag="a")
                # psum banks: f | i,g (merged for one sigmoid) | o
                # (start=True clears has_written for the WHOLE bank, so each
                # accumulation group needs its own bank)
                pf = psg.tile([128, gw], fp32, tag="pf", name="pf")
                pig = psg.tile([128, 2 * gw], fp32, tag="pig", name="pig")
                po = psg.tile([128, gw], fp32, tag="po", name="po")

                def gate_mms(ps, mlo, mhi):
                    for m in range(mlo, mhi):
                        for k in range(KT):
                            nc.tensor.matmul(
                                ps[:, (m - mlo) * BL:(m - mlo + 1) * BL],
                                whh[:, k, m * 128:(m + 1) * 128],
                                h8[:, k, t * BL:(t + 1) * BL],
                                start=False,
                                stop=(m == mhi - 1 and k == KT - 1))

                # i,g first: sigma(i,g) anchors the serial spine, so its
                # matmuls must finish as early as possible.  Seeds adjacent
                # (identical ident lhsT back-to-back: probes LDW dedup).
                nc.tensor.matmul(pig[:], ident[:],
                                 ring[:, 4:12, s * BL:(s + 1) * BL],
                                 start=True, stop=False)
                nc.tensor.matmul(pf[:], ident[:],
                                 ring[:, 0:4, s * BL:(s + 1) * BL],
                                 start=True, stop=False)
                nc.tensor.matmul(po[:], ident[:],
                                 ring[:, 12:16, s * BL:(s + 1) * BL],
                                 start=True, stop=False)
                gate_mms(pig, 4, 12)
                gate_mms(pf, 0, 4)
                gate_mms(po, 12, 16)

                # ACT order: sig(ig) -> sig(f) -> sig(o) -> sig(2c); sig(o)
                # fills the ACT gap between sig(f) and sig(2c)
                nc.scalar.activation(a[:, gw:3 * gw], pig[:], Act.Sigmoid,
                                     scale=1.0 / WS)
                nc.scalar.activation(a[:, 0:gw], pf[:], Act.Sigmoid,
                                     scale=1.0 / WS)
                nc.scalar.activation(a[:, 3 * gw:4 * gw], po[:], Act.Sigmoid,
                                     scale=1.0 / WS)

                # DVE spine: t1h first (its dep sig(ig) lands before sig(f))
                t1h = wk.tile([128, gw], fp32, tag="t1h")
                nc.vector.scalar_tensor_tensor(
                    t1h[:], a[:, 2 * gw:3 * gw], -0.5, a[:, gw:2 * gw],
                    Alu.add, Alu.mult)
                if t > 0:
                    c1 = wk.tile([128, gw], fp32, tag="c1")
                    nc.vector.tensor_mul(c1[:], a[:, 0:gw], c_prev[:])
                c_new = cp.tile([128, gw], fp32, tag="c")
                if t == 0:
                    nc.vector.tensor_scalar_mul(c_new[:], t1h[:], 2.0)
                else:
                    nc.vector.scalar_tensor_tensor(
                        c_new[:], t1h[:], 2.0, c1[:], Alu.mult, Alu.add)
                u = wk.tile([128, gw], fp32, tag="u")
                nc.scalar.activation(u[:], c_new[:], Act.Sigmoid, scale=2.0)
                u_r = u[:].rearrange("p (k b) -> p k b", b=BL)
                o_r = a[:, 3 * gw:4 * gw].rearrange("p (k b) -> p k b", b=BL)
                nc.vector.scalar_tensor_tensor(
                    h8[:, :, (t + 1) * BL:(t + 2) * BL], u_r, -0.5, o_r,
                    Alu.add, Alu.mult)
                nc.vector.scalar_tensor_tensor(
                    hb[:, :, (t + 1) * BL:(t + 2) * BL], u_r, -0.5, o_r,
                    Alu.add, Alu.mult)
                c_prev = c_new

                # stuffers AFTER the gate MMs: they run inside the spine
                # window on the in-order PE instead of delaying the step
                if ch + lead < nch:
                    # produce chunk ch+1 at 1.25x consumption rate so it
                    # completes ~80% into chunk ch (jitter margin)
                    tgt = 4 * MT * ch + min(4 * MT, (s + 1) * 5 * MT // spc)
                    while xg_done < tgt:
                        u_i = xg_done % (4 * MT)
                        xg_mm(ch + lead, u_i // KT, u_i % KT)
                        xg_done += 1
                if t >= spc:
                    tgt = min(n_fc_mm, 4 * KT * (t // spc),
                              ((t - spc) * 4) // 6 + 2)
                    while fc_done < tgt:
                        u_i = fc_done
                        fc_mm(u_i // (KT * (NOUT // 128)),
                              (u_i // KT) % (NOUT // 128), u_i % KT)
                        fc_done += 1

                if ch - 1 in rings and s == spc - 1:
                    del rings[ch - 1]

            while fc_done < n_fc_mm:  # FC epilogue
                u_i = fc_done
                fc_mm(u_i // (KT * (NOUT // 128)), (u_i // KT) % (NOUT // 128),
                      u_i % KT)
                fc_done += 1

    nc.compile()
    return nc


def _get_program(t_steps=T):
    if t_steps not in _CACHE:
        _CACHE[t_steps] = _build_program(t_steps)
    return _CACHE[t_steps]


def _to_bf16(arr):
    import ml_dtypes

    return np.asarray(arr).astype(ml_dtypes.bfloat16)


def _to_fp8(arr):
    import ml_dtypes

    return np.asarray(arr).astype(ml_dtypes.float8_e4m3fn)


def _prep_weight_T(w_gate_rows, conv):
    """[rows, 512] (gate-permuted rows) -> lhsT layout [128, KT, rows]."""
    wt = np.ascontiguousarray(np.asarray(w_gate_rows, np.float32).T)
    return conv(wt.reshape(KT, 128, wt.shape[1]).transpose(1, 0, 2))


def _gate_perm_rows(w):
    blocks = np.split(np.asarray(w, np.float32), 4, axis=0)
    return np.concatenate([blocks[i] for i in GATE_PERM], axis=0)


def _g_row_scale(rows_scaled):
    """Scale the g-gate block (3rd group in [f,i,g,o] order) by 2."""
    out = rows_scaled.copy()
    out[2 * H:3 * H] *= 2.0
    return out


def _make_in_maps(x, w_ih_f, w_hh_f, b_ih_f, b_hh_f, w_ih_b, w_hh_b, b_ih_b,
                  b_hh_b, fc_w, fc_b, t_steps):
    per_dir = []
    for d, (wih, whh, bih, bhh) in enumerate(
        [(w_ih_f, w_hh_f, b_ih_f, b_hh_f), (w_ih_b, w_hh_b, b_ih_b, b_hh_b)]
    ):
        # [f,i,g,o] rows; xg path x WS (g-rows x2 more); recurrent weights
        # additionally x2 (stored state is h/2) -> x(2*WS), fp8
        wih_r = _g_row_scale(_gate_perm_rows(wih) * WS)
        whh_r = _g_row_scale(_gate_perm_rows(whh) * (2.0 * WS))
        bias_r = _g_row_scale(
            _gate_perm_rows(
                (np.asarray(bih) + np.asarray(bhh))[:, None]) * WS)[:, 0]
        per_dir.append({
            "wihT": _prep_weight_T(wih_r, _to_bf16),
            "whhT": _prep_weight_T(whh_r, _to_fp8),
            "fcwT": _prep_weight_T(np.ascontiguousarray(
                np.asarray(fc_w, np.float32)[:, d * H:(d + 1) * H]) * 2.0,
                _to_bf16),
            "bias": np.ascontiguousarray(
                bias_r.reshape(MT, 128).T).astype(np.float32),
        })
    in_maps = []
    for c in range(8):
        d, q = c // 4, c % 4
        xq = np.asarray(x)[:t_steps, q * BL:(q + 1) * BL, :]
        if d == 1:
            xq = xq[::-1]
        xT = xq.transpose(2, 0, 1).reshape(KT, 128, t_steps * BL).transpose(1, 0, 2)
        m = dict(per_dir[d])
        m["xT"] = _to_bf16(xT)
        in_maps.append(m)
    return in_maps


def _assemble(results, fc_b, t_steps):
    out = np.zeros((t_steps, B, NOUT), np.float32)
    for c in range(8):
        d, q = c // 4, c % 4
        oT = np.asarray(results[c]["outT"]).reshape(NOUT, t_steps, BL)
        part = oT.transpose(1, 2, 0)  # [t, b, out]
        if d == 1:
            part = part[::-1]
        out[:, q * BL:(q + 1) * BL, :] += part
    out += np.asarray(fc_b, np.float32)
    return out


def kernel(x, w_ih_f, w_hh_f, b_ih_f, b_hh_f, w_ih_b, w_hh_b, b_ih_b, b_hh_b,
           fc_w, fc_b, _t_steps=T, _trace=False, _trace_kwargs=None):
    from concourse.bass_utils import run_bass_kernel_spmd

    nc = _get_program(_t_steps)
    in_maps = _make_in_maps(x, w_ih_f, w_hh_f, b_ih_f, b_hh_f, w_ih_b, w_hh_b,
                            b_ih_b, b_hh_b, fc_w, fc_b, _t_steps)
    res = run_bass_kernel_spmd(
        nc, in_maps, core_ids=list(range(8)), trace=_trace,
        **(_trace_kwargs or {}),
    )
    out = _assemble(res.results, fc_b, _t_steps)
    if _trace:
        kernel._last_result = res
    return out


# revision 21
# speedup vs baseline: 1.7452x; 1.0012x over previous
"""BiLSTM (T=256, B=64, NIN=H=NOUT=512) Trainium2 kernel over 8 NeuronCores.

Sharding: direction (2) x batch-quarter (4) = 8 cores, SPMD (one program).
Each core runs one direction's LSTM for 16 batch rows (backward cores get
time-reversed x), then computes its half of the final FC:
    out = h_f @ fc_w[:, :H].T + h_b @ fc_w[:, H:].T + fc_b
The host sums the two partial FC outputs per batch quarter. No collectives.

V2 changes vs baseline:
  - Recurrence matmuls in fp8(e4m3) with DoubleRow perf mode: contraction
    256 per instruction -> 32 gate MMs/step instead of 64. Weights scaled
    x32 (avoids e4m3 subnormals); the sigmoid ACT op applies scale=1/32.
  - Sigmoid-only cell math (no tanh table):
      * g-gate rows pre-scaled x2 so sigma(2 zg) = (tanh(zg)+1)/2
      * stored state h' = h/2 with W_hh / fc_w pre-scaled x2
      * t1h = (Gg-0.5)*Gi, c = 2*t1h + Gf*c_prev, u = sigma(2c),
        h' = (u-0.5)*Go  -- each one fused scalar_tensor_tensor DVE op.
  - PSUM layout [f][i|g][o]: i,g share one bank so sigma(i,g) is a single
    merged ACT op (the spine's first activation).
  - h' written twice: fp8 copy for the recurrence rhs, bf16 copy for FC
    (fp8 h into FC fails the accuracy budget; bf16 copy passes at 7e-3).
  - xg/FC stuffer matmuls emitted AFTER each step's gate MMs so they fill
    the ACT/DVE spine window instead of delaying the gate matmuls.
"""

import numpy as np

T, B, NIN, H, NOUT = 256, 64, 512, 512, 512
BL = B // 4          # local batch per core (batch quarter)
KT = H // 128        # 4 k-tiles over the hidden/contraction dim
MT = (4 * H) // 128  # 16 m-tiles over the gate dim
# PyTorch gate blocks [i,f,g,o] -> our order [f,i,g,o]
GATE_PERM = [1, 0, 2, 3]
WS = 32.0            # fp8 weight scale (ACT de-scales with 1/WS)

_CACHE = {}


def _build_program(t_steps):
    import concourse.mybir as mybir
    import concourse.tile as tile
    from concourse import bacc
    from concourse.masks import make_identity

    fp32 = mybir.dt.float32
    bf16 = mybir.dt.bfloat16
    fp8 = mybir.dt.float8e4
    Act = mybir.ActivationFunctionType
    Alu = mybir.AluOpType
    DR = mybir.MatmulPerfMode.DoubleRow

    ntb = t_steps * BL
    chunk = min(512, ntb)
    nch = ntb // chunk
    spc = chunk // BL   # steps per chunk
    lead = min(1, nch)  # xg chunks computed ahead (chunk 0 only upfront)

    nc = bacc.Bacc("TRN2", target_bir_lowering=False, debug=False)
    xT_d = nc.dram_tensor("xT", [128, KT, ntb], bf16, kind="ExternalInput")
    wih_d = nc.dram_tensor("wihT", [128, KT, 4 * H], bf16, kind="ExternalInput")
    whh_d = nc.dram_tensor("whhT", [128, KT, 4 * H], fp8, kind="ExternalInput")
    fcw_d = nc.dram_tensor("fcwT", [128, KT, NOUT], bf16, kind="ExternalInput")
    bias_d = nc.dram_tensor("bias", [128, MT], fp32, kind="ExternalInput")
    outT_d = nc.dram_tensor("outT", [NOUT // 128, 128, ntb], fp32, kind="ExternalOutput")

    gw = KT * BL  # 64 columns per gate group

    with tile.TileContext(nc) as tc:
        with (
            tc.tile_pool(name="weights", bufs=1) as wp,
            tc.tile_pool(name="state", bufs=1) as sp,
            tc.tile_pool(name="ring", bufs=lead + 1) as rp,
            tc.tile_pool(name="stage", bufs=3) as stp,
            tc.tile_pool(name="work", bufs=2) as wk,
            tc.tile_pool(name="cpool", bufs=2) as cp,
            tc.tile_pool(name="psg", bufs=2, space="PSUM") as psg,
            tc.tile_pool(name="psb", bufs=2, space="PSUM") as psb,
        ):
            xT = wp.tile([128, KT, ntb], bf16)
            wih = wp.tile([128, KT, 4 * H], bf16)
            whh = wp.tile([128, KT, 4 * H], fp8)
            fcw = wp.tile([128, KT, NOUT], bf16)
            bias = wp.tile([128, MT], fp32)
            ident = wp.tile([128, 128], bf16)
            # fp8 copy feeds the recurrence matmuls, bf16 copy feeds the FC
            h8 = sp.tile([128, KT, (t_steps + 1) * BL], fp8)
            hb = sp.tile([128, KT, (t_steps + 1) * BL], bf16)

            # weights + chunk 0 first: the prologue xg matmuls need them
            nc.sync.dma_start(xT[:, :, 0:chunk], xT_d[:, :, 0:chunk])
            nc.sync.dma_start(wih[:], wih_d[:])
            nc.sync.dma_start(bias[:], bias_d[:])
            nc.sync.dma_start(whh[:], whh_d[:])
            nc.sync.dma_start(fcw[:], fcw_d[:])
            for ch in range(1, nch):
                nc.sync.dma_start(xT[:, :, ch * chunk:(ch + 1) * chunk],
                                  xT_d[:, :, ch * chunk:(ch + 1) * chunk])
            make_identity(nc, ident[:])
            nc.vector.memset(h8[:, :, 0:BL], 0.0)

            rings = {}
            xg_ps = {}
            fc_ps = [None]

            def get_ring(ch):
                if ch not in rings:
                    rings[ch] = rp.tile([128, MT, chunk], bf16, tag="ring",
                                        name=f"ring{ch}")
                return rings[ch]

            def xg_mm(ch, m, k):
                """One k-MM of the xg unit (ch, m); evacuates on k==KT-1."""
                ring = get_ring(ch)
                if k == 0:
                    xg_ps[(ch, m)] = psb.tile([128, chunk], fp32, tag="big",
                                              name=f"xgps{ch}_{m}")
                ps = xg_ps[(ch, m)]
                nc.tensor.matmul(
                    ps[:], wih[:, k, m * 128:(m + 1) * 128],
                    xT[:, k, ch * chunk:(ch + 1) * chunk],
                    start=(k == 0), stop=(k == KT - 1))
                if k == KT - 1:
                    # two half-evacuations bound the damage if the list
                    # scheduler slots one into the DVE spine
                    hc = chunk // 2
                    nc.vector.tensor_scalar_add(ring[:, m, 0:hc], ps[:, 0:hc],
                                                bias[:, m:m + 1])
                    nc.vector.tensor_scalar_add(ring[:, m, hc:chunk],
                                                ps[:, hc:chunk],
                                                bias[:, m:m + 1])
                    del xg_ps[(ch, m)]

            FCH = min(256, ntb)  # fc works in half-chunks: ready 16 steps
            spch = FCH // BL     # earlier, and a smaller epilogue tail

            def fc_mm(hc, m, k):
                if k == 0:
                    fc_ps[0] = psb.tile([128, FCH], fp32, tag="big",
                                        name=f"fcps{m}_{hc}",
                                        padded_shape=[128, chunk])
                ps = fc_ps[0]
                nc.tensor.matmul(
                    ps[:], fcw[:, k, m * 128:(m + 1) * 128],
                    hb[:, k, BL + hc * FCH:BL + (hc + 1) * FCH],
                    start=(k == 0), stop=(k == KT - 1))
                if k == KT - 1:
                    st = stp.tile([128, FCH], fp32, tag="ost",
                                  padded_shape=[128, chunk])
                    nc.vector.tensor_copy(st[:], ps[:])
                    nc.sync.dma_start(
                        outT_d[m, :, hc * FCH:(hc + 1) * FCH], st[:])

            # prologue: first `lead` xg chunks
            for ch in range(lead):
                for m in range(MT):
                    for k in range(KT):
                        xg_mm(ch, m, k)

            n_fc_mm = (NOUT // 128) * (ntb // FCH) * KT
            fc_done = 0
            xg_done = 0  # MMs emitted for chunks >= lead
            c_prev = None
            for t in range(t_steps):
                s = t % spc
                ch = t // spc
                ring = get_ring(ch)

                a = wk.tile([128, 4 * gw], fp32, tag="a")
                # psum banks: f | i,g (merged for one sigmoid) | o
                # (start=True clears has_written for the WHOLE bank, so each
                # accumulation group needs its own bank)
                pf = psg.tile([128, gw], fp32, tag="pf", name="pf")
                pig = psg.tile([128, 2 * gw], fp32, tag="pig", name="pig")
                po = psg.tile([128, gw], fp32, tag="po", name="po")

                def gate_mms(ps, mlo, mhi):
                    for m in range(mlo, mhi):
                        for k in range(KT):
                            nc.tensor.matmul(
                                ps[:, (m - mlo) * BL:(m - mlo + 1) * BL],
                                whh[:, k, m * 128:(m + 1) * 128],
                                h8[:, k, t * BL:(t + 1) * BL],
                                start=False,
                                stop=(m == mhi - 1 and k == KT - 1))

                # i,g first: sigma(i,g) anchors the serial spine, so its
                # matmuls must finish as early as possible.  Seeds adjacent
                # (identical ident lhsT back-to-back: probes LDW dedup).
                nc.tensor.matmul(pig[:], ident[:],
                                 ring[:, 4:12, s * BL:(s + 1) * BL],
                                 start=True, stop=False)
                nc.tensor.matmul(pf[:], ident[:],
                                 ring[:, 0:4, s * BL:(s + 1) * BL],
                                 start=True, stop=False)
                nc.tensor.matmul(po[:], ident[:],
                                 ring[:, 12:16, s * BL:(s + 1) * BL],
                                 start=True, stop=False)
                gate_mms(pig, 4, 12)
                gate_mms(pf, 0, 4)
                gate_mms(po, 12, 16)

                # ACT order: sig(ig) -> sig(f) -> sig(o) -> sig(2c); sig(o)
                # fills the ACT gap between sig(f) and sig(2c)
                nc.scalar.activation(a[:, gw:3 * gw], pig[:], Act.Sigmoid,
                                     scale=1.0 / WS)
                nc.scalar.activation(a[:, 0:gw], pf[:], Act.Sigmoid,
                                     scale=1.0 / WS)
                nc.scalar.activation(a[:, 3 * gw:4 * gw], po[:], Act.Sigmoid,
                                     scale=1.0 / WS)

                # DVE spine: t1h first (its dep sig(ig) lands before sig(f))
                t1h = wk.tile([128, gw], fp32, tag="t1h")
                nc.vector.scalar_tensor_tensor(
                    t1h[:], a[:, 2 * gw:3 * gw], -0.5, a[:, gw:2 * gw],
                    Alu.add, Alu.mult)
                if t > 0:
                    c1 = wk.tile([128, gw], fp32, tag="c1")
                    nc.vector.tensor_mul(c1[:], a[:, 0:gw], c_prev[:])
                c_new = cp.tile([128, gw], fp32, tag="c")
                if t == 0:
                    nc.vector.tensor_scalar_mul(c_new[:], t1h[:], 2.0)
                else:
                    nc.vector.scalar_tensor_tensor(
                        c_new[:], t1h[:], 2.0, c1[:], Alu.mult, Alu.add)
                u = wk.tile([128, gw], fp32, tag="u")
                nc.scalar.activation(u[:], c_new[:], Act.Sigmoid, scale=2.0)
                u_r = u[:].rearrange("p (k b) -> p k b", b=BL)
                o_r = a[:, 3 * gw:4 * gw].rearrange("p (k b) -> p k b", b=BL)
                nc.vector.scalar_tensor_tensor(
                    h8[:, :, (t + 1) * BL:(t + 2) * BL], u_r, -0.5, o_r,
                    Alu.add, Alu.mult)
                nc.vector.scalar_tensor_tensor(
                    hb[:, :, (t + 1) * BL:(t + 2) * BL], u_r, -0.5, o_r,
                    Alu.add, Alu.mult)
                c_prev = c_new

                # stuffers AFTER the gate MMs: they run inside the spine
                # window on the in-order PE instead of delaying the step
                if ch + lead < nch:
                    # produce chunk ch+1 at 1.25x consumption rate so it
                    # completes ~80% into chunk ch (jitter margin)
                    tgt = 4 * MT * ch + min(4 * MT, (s + 1) * 5 * MT // spc)
                    while xg_done < tgt:
                        u_i = xg_done % (4 * MT)
                        xg_mm(ch + lead, u_i // KT, u_i % KT)
                        xg_done += 1
                if t >= spch:
                    tgt = min(n_fc_mm, 4 * KT * (t // spch),
                              ((t - spch) * n_fc_mm) // max(1, t_steps - spch - 8) + 2)
                    while fc_done < tgt:
                        u_i = fc_done
                        fc_mm(u_i // (KT * (NOUT // 128)),
                              (u_i // KT) % (NOUT // 128), u_i % KT)
                        fc_done += 1

                if ch - 1 in rings and s == spc - 1:
                    del rings[ch - 1]

            while fc_done < n_fc_mm:  # FC epilogue
                u_i = fc_done
                fc_mm(u_i // (KT * (NOUT // 128)), (u_i // KT) % (NOUT // 128),
                      u_i % KT)
                fc_done += 1

    nc.compile()
    return nc


def _get_program(t_steps=T):
    if t_steps not in _CACHE:
        _CACHE[t_steps] = _build_program(t_steps)
    return _CACHE[t_steps]


def _to_bf16(arr):
    import ml_dtypes

    return np.asarray(arr).astype(ml_dtypes.bfloat16)


def _to_fp8(arr):
    import ml_dtypes

    return np.asarray(arr).astype(ml_dtypes.float8_e4m3fn)


def _prep_weight_T(w_gate_rows, conv):
    """[rows, 512] (gate-permuted rows) -> lhsT layout [128, KT, rows]."""
    wt = np.ascontiguousarray(np.asarray(w_gate_rows, np.float32).T)
    return conv(wt.reshape(KT, 128, wt.shape[1]).transpose(1, 0, 2))


def _gate_perm_rows(w):
    blocks = np.split(np.asarray(w, np.float32), 4, axis=0)
    return np.concatenate([blocks[i] for i in GATE_PERM], axis=0)


def _g_row_scale(rows_scaled):
    """Scale the g-gate block (3rd group in [f,i,g,o] order) by 2."""
    out = rows_scaled.copy()
    out[2 * H:3 * H] *= 2.0
    return out


def _make_in_maps(x, w_ih_f, w_hh_f, b_ih_f, b_hh_f, w_ih_b, w_hh_b, b_ih_b,
                  b_hh_b, fc_w, fc_b, t_steps):
    per_dir = []
    for d, (wih, whh, bih, bhh) in enumerate(
        [(w_ih_f, w_hh_f, b_ih_f, b_hh_f), (w_ih_b, w_hh_b, b_ih_b, b_hh_b)]
    ):
        # [f,i,g,o] rows; xg path x WS (g-rows x2 more); recurrent weights
        # additionally x2 (stored state is h/2) -> x(2*WS), fp8
        wih_r = _g_row_scale(_gate_perm_rows(wih) * WS)
        whh_r = _g_row_scale(_gate_perm_rows(whh) * (2.0 * WS))
        bias_r = _g_row_scale(
            _gate_perm_rows(
                (np.asarray(bih) + np.asarray(bhh))[:, None]) * WS)[:, 0]
        per_dir.append({
            "wihT": _prep_weight_T(wih_r, _to_bf16),
            "whhT": _prep_weight_T(whh_r, _to_fp8),
            "fcwT": _prep_weight_T(np.ascontiguousarray(
                np.asarray(fc_w, np.float32)[:, d * H:(d + 1) * H]) * 2.0,
                _to_bf16),
            "bias": np.ascontiguousarray(
                bias_r.reshape(MT, 128).T).astype(np.float32),
        })
    in_maps = []
    for c in range(8):
        d, q = c // 4, c % 4
        xq = np.asarray(x)[:t_steps, q * BL:(q + 1) * BL, :]
        if d == 1:
            xq = xq[::-1]
        xT = xq.transpose(2, 0, 1).reshape(KT, 128, t_steps * BL).transpose(1, 0, 2)
        m = dict(per_dir[d])
        m["xT"] = _to_bf16(xT)
        in_maps.append(m)
    return in_maps


def _assemble(results, fc_b, t_steps):
    out = np.zeros((t_steps, B, NOUT), np.float32)
    for c in range(8):
        d, q = c // 4, c % 4
        oT = np.asarray(results[c]["outT"]).reshape(NOUT, t_steps, BL)
        part = oT.transpose(1, 2, 0)  # [t, b, out]
        if d == 1:
            part = part[::-1]
        out[:, q * BL:(q + 1) * BL, :] += part
    out += np.asarray(fc_b, np.float32)
    return out


def kernel(x, w_ih_f, w_hh_f, b_ih_f, b_hh_f, w_ih_b, w_hh_b, b_ih_b, b_hh_b,
           fc_w, fc_b, _t_steps=T, _trace=False, _trace_kwargs=None):
    from concourse.bass_utils import run_bass_kernel_spmd

    nc = _get_program(_t_steps)
    in_maps = _make_in_maps(x, w_ih_f, w_hh_f, b_ih_f, b_hh_f, w_ih_b, w_hh_b,
                            b_ih_b, b_hh_b, fc_w, fc_b, _t_steps)
    res = run_bass_kernel_spmd(
        nc, in_maps, core_ids=list(range(8)), trace=_trace,
        **(_trace_kwargs or {}),
    )
    out = _assemble(res.results, fc_b, _t_steps)
    if _trace:
        kernel._last_result = res
    return out
